# revision 24
# baseline (speedup 1.0000x reference)
"""Trainium2 Bass kernel for nn_FLASH_40458591928592 (sparse_attention).

Sequence-sharded over 8 NeuronCores: 1024 tokens (= 4 groups of 256) per core.
Mixed precision, validated against a numpy e4m3 simulation (rel 1.66e-2 < 2e-2):
  qk GEMM : fully fp8 DoubleRow (x*0.25 stationary-free scales, W*4)
  v GEMM  : fully fp8 DoubleRow; v_h and tails stored fp8 (storage only)
  gate    : k-tiles 0-1 of 8 fp8 DR, rest bf16 (same PSUM, products at scale 1)
  y       : HID k-tiles 0-3 of 16 fp8 DR (og et 0-3 written fp8*0.25 by DVE)

Phase order keeps the PE dense and the HAM clock warm:
  dummy warmup matmuls through the DMA lead-in -> v -> qk(3,2,0,1) -> sim/attn
  -> lk transposes -> kv chains (AllGather per e-half fires ~60us) -> quad+conv
  (overlapping the collectives) -> lin -> gate+y per stream.
One LDWEIGHTS feeds 2-4 matmuls everywhere (ec/ch/th/nch pairing).
SBUF is tag-chained across serial phases (vh->ogb, wv8->woutb, qkT->gt,
S_full->og8, t_half->wg8, lk->wout8, S_offb->xr, tails->ysb).
"""

from contextlib import ExitStack

import numpy as np
import ml_dtypes

import concourse.tile as tile
from concourse import bacc, mybir
from concourse.bass_utils import run_bass_kernel_spmd
from concourse.masks import make_identity

BF = mybir.dt.bfloat16
F8 = mybir.dt.float8e4
F32 = mybir.dt.float32
bf16 = ml_dtypes.bfloat16
fp8 = ml_dtypes.float8_e4m3
DRM = mybir.MatmulPerfMode.DoubleRow

G = 256
DIM = 1024
HID = 2048
DQK = 128
NSEQ = 8192
NC = 8
T = NSEQ // NC        # 1024 tokens per core
NG = T // G           # 4 groups per core
KD = DIM // 128       # 8 k-tiles over dim
ET = HID // 128       # 16 e-tiles over hid
TT = T // 128         # 8 token tiles
EH = HID // 2         # 1024 cols per e-half

KG8 = 2               # gate fp8 k-tiles (of KD); one DR pair
KY8 = 4               # y fp8 k-tiles (of ET); must be even
KB = KD - KG8         # bf16 k-tiles for gate
XS = 0.25             # fp8 x-side scale
WS = 4.0              # fp8 w-side scale (XS*WS == 1 -> shared-PSUM)
NWARM = 18            # HAM warmup dummy matmuls

AF = mybir.ActivationFunctionType
ALU = mybir.AluOpType

DEBUG_DUMPS = False
WITH_VBIAS = True
WITH_OBIAS = True


def _emit(tc, ap):
    nc = tc.nc
    with ExitStack() as ctx:
        pass

        consts = ctx.enter_context(tc.tile_pool(name="consts", bufs=1))
        p_x8 = ctx.enter_context(tc.tile_pool(name="x8", bufs=2))
        p_xtb = ctx.enter_context(tc.tile_pool(name="xtb", bufs=3))
        p_x08 = ctx.enter_context(tc.tile_pool(name="x08", bufs=1))
        p_big = ctx.enter_context(tc.tile_pool(name="big", bufs=1))   # vh8 -> ogb
        p_qog = ctx.enter_context(tc.tile_pool(name="qog", bufs=1))   # qkT -> gt
        p_lk = ctx.enter_context(tc.tile_pool(name="lk", bufs=1))     # lk -> wout8
        p_w1 = ctx.enter_context(tc.tile_pool(name="w1", bufs=1))     # wv8 -> woutb
        p_w2 = ctx.enter_context(tc.tile_pool(name="w2", bufs=1))     # wgb
        p_tails = ctx.enter_context(tc.tile_pool(name="tails", bufs=1))  # -> ysb
        p_so = ctx.enter_context(tc.tile_pool(name="so", bufs=2))     # t_half -> wg8
        p_sob = ctx.enter_context(tc.tile_pool(name="sob", bufs=2))   # xr
        p_sf = ctx.enter_context(tc.tile_pool(name="sf", bufs=1))     # S_full -> og8
        p_a0 = ctx.enter_context(tc.tile_pool(name="a0", bufs=4))
        p_a1 = ctx.enter_context(tc.tile_pool(name="a1", bufs=4))
        p_outT = ctx.enter_context(tc.tile_pool(name="outT", bufs=1))
        ps1 = ctx.enter_context(tc.tile_pool(name="ps1", bufs=3, space="PSUM"))
        ps2 = ctx.enter_context(tc.tile_pool(name="ps2", bufs=2, space="PSUM"))

        # warm-up collective first: its ~40us post-trigger setup runs in the
        # shadow of the input DMAs, so the real AllGathers start immediately
        cwarm = consts.tile([128, 16], BF, tag="cwarm")
        nc.vector.memset(cwarm, 0.0)
        nc.sync.dma_start(ap["cc_warm_in"], cwarm)
        nc.gpsimd.collective_compute(
            "AllGather", ALU.bypass, replica_groups=[list(range(NC))],
            ins=[ap["cc_warm_in"]], outs=[ap["cc_warm_out"]])

        # ---- HAM warmup: keep PE busy through the DMA lead-in ----
        ident = consts.tile([128, 128], BF, tag="ident")
        make_identity(nc, ident)
        for _ in range(NWARM):
            pw = ps2.tile([128, 128], F32, tag="ps2", name="pw")
            nc.tensor.matmul(pw, ident, ident, start=True, stop=True)

        # ---- first DMAs: v-GEMM inputs, then qk weights ----
        x08 = p_x08.tile([128, KD, T], F8, tag="x08")
        wv8 = p_w1.tile([128, KD, HID], F8, tag="w1")
        for q in range(4):
            nc.sync.dma_start(
                x08[:, q * 2:(q + 1) * 2, :],
                ap["xt08"][q * 256:(q + 1) * 256, :].rearrange(
                    "(kt p) t -> p kt t", p=128))
            eng = nc.scalar if q < 2 else nc.gpsimd
            eng.dma_start(
                wv8[:, q * 2:(q + 1) * 2, :],
                ap["wv8"][q * 256:(q + 1) * 256, :].rearrange(
                    "(kt p) e -> p kt e", p=128))
        bqk = consts.tile([128, 1], F32, tag="bqk")
        nc.scalar.dma_start(bqk, ap["bqk"])
        wqk8 = consts.tile([128, KD, DQK], F8, tag="wqk8")
        nc.scalar.dma_start(wqk8, ap["wqk8"].rearrange("(kt p) q -> p kt q", p=128))
        xh8 = consts.tile([128, KD, 32], F8, tag="xh8")
        nc.scalar.dma_start(xh8, ap["xh8"].rearrange("(kt p) t -> p kt t", p=128))

        x8f = {0: x08}

        def load_x8f(s):
            t8 = p_x8.tile([128, KD, T], F8, tag="x8", name=f"x8_{s}")
            nc.sync.dma_start(t8, ap["xt8"][s - 1].rearrange("(kt p) t -> p kt t",
                                                             p=128))
            x8f[s] = t8

        load_x8f(3)
        load_x8f(2)

        # remaining consts (DMA behind weights on scalar queue)
        triu = consts.tile([128, 128], BF, tag="triu")
        nc.scalar.dma_start(triu, ap["triu"])
        bdiag = consts.tile([128, 128], BF, tag="bdiag")
        nc.scalar.dma_start(bdiag, ap["bdiag"])
        bcorn = consts.tile([128, 128], BF, tag="bcorn")
        nc.scalar.dma_start(bcorn, ap["bcorn"])
        bprev = consts.tile([32, 256], BF, tag="bprev")
        nc.scalar.dma_start(bprev, ap["bprev"])
        hmask = consts.tile([32, 1], F32, tag="hmask")
        nc.scalar.dma_start(hmask, ap["hmask"])
        wsumw = consts.tile([128, NC], F32, tag="wsumw")
        nc.scalar.dma_start(wsumw, ap["wsumw"])
        bgate = consts.tile([128, ET], F32, tag="bgate")
        nc.scalar.dma_start(bgate, ap["bgate"])
        if WITH_VBIAS or WITH_OBIAS:
            ones_t = consts.tile([1, 1024], BF, tag="ones")
            nc.vector.memset(ones_t, 1.0)
        if WITH_VBIAS:
            wvb = consts.tile([1, HID], BF, tag="wvb")
            nc.scalar.dma_start(wvb, ap["wvb"])
        if WITH_OBIAS:
            bout = consts.tile([1, DIM], BF, tag="bout")
            nc.scalar.dma_start(bout, ap["bout"])

        # ---- v GEMM: fp8 DR, one xt-pair LDWEIGHTS feeds 4 e-chunks ----
        v_h = p_big.tile([128, TT, HID], F8, tag="big", name="v_h")
        for tt in range(TT):
            pv = [ps1.tile([128, 1024], F32, tag="ps1", name="pv")
                  for _ in range(2)]
            for kp in range(KD // 2):
                for ec in range(4):
                    nc.tensor.matmul(
                        pv[ec // 2][:, (ec % 2) * 512:(ec % 2 + 1) * 512],
                        x08[:, 2 * kp:2 * kp + 2, tt * 128:(tt + 1) * 128],
                        wv8[:, 2 * kp:2 * kp + 2, ec * 512:(ec + 1) * 512],
                        start=(kp == 0),
                        stop=(kp == KD // 2 - 1 and not WITH_VBIAS),
                        perf_mode=DRM, skip_group_check=True)
            if WITH_VBIAS:
                for ec in range(4):
                    nc.tensor.matmul(pv[ec // 2][:, (ec % 2) * 512:(ec % 2 + 1) * 512],
                                     ones_t[0:1, 0:128],
                                     wvb[0:1, ec * 512:(ec + 1) * 512],
                                     start=False, stop=True, skip_group_check=True)
            for eh in range(2):
                nc.scalar.activation(v_h[:, tt, eh * 1024:(eh + 1) * 1024], pv[eh],
                                     AF.Silu, bias=0.0, scale=1.0)

        # halo: last 32 tokens of the previous core (masked on core 0)
        tails = p_tails.tile([32, NG, HID], F8, tag="tails")
        ph = [ps1.tile([32, 1024], F32, tag="ps1", name="ph") for _ in range(2)]
        for kp in range(KD // 2):
            for ec in range(4):
                nc.tensor.matmul(
                    ph[ec // 2][:, (ec % 2) * 512:(ec % 2 + 1) * 512],
                    xh8[:, 2 * kp:2 * kp + 2, :],
                    wv8[:, 2 * kp:2 * kp + 2, ec * 512:(ec + 1) * 512],
                    start=(kp == 0), stop=(kp == KD // 2 - 1 and not WITH_VBIAS),
                    perf_mode=DRM, skip_group_check=True)
        if WITH_VBIAS:
            for ec in range(4):
                nc.tensor.matmul(ph[ec // 2][:, (ec % 2) * 512:(ec % 2 + 1) * 512],
                                 ones_t[0:1, 0:32],
                                 wvb[0:1, ec * 512:(ec + 1) * 512],
                                 start=False, stop=True, skip_group_check=True)
        for eh in range(2):
            nc.scalar.activation(tails[:, 0, eh * 1024:(eh + 1) * 1024], ph[eh],
                                 AF.Silu, bias=0.0, scale=1.0)
            nc.vector.tensor_scalar_mul(tails[:, 0, eh * 1024:(eh + 1) * 1024],
                                        tails[:, 0, eh * 1024:(eh + 1) * 1024],
                                        hmask)
        for g in range(1, NG):
            nc.scalar.dma_start(tails[:, g, :], v_h[96:128, 2 * g - 1, :])

        # ---- qk streams: fully fp8 DR, ch-paired ----
        qkT = p_qog.tile([128, 4, T], BF, tag="qog", name="qkT")

        def qk_stream(s):
            pc = ps1.tile([128, 1024], F32, tag="ps1", name="pc")
            for kp in range(KD // 2):
                for ch in range(2):
                    nc.tensor.matmul(pc[:, ch * 512:(ch + 1) * 512],
                                     wqk8[:, 2 * kp:2 * kp + 2, :],
                                     x8f[s][:, 2 * kp:2 * kp + 2,
                                            ch * 512:(ch + 1) * 512],
                                     start=(kp == 0), stop=(kp == KD // 2 - 1),
                                     perf_mode=DRM, skip_group_check=True)
            nc.scalar.activation(qkT[:, s, :], pc, AF.Silu, bias=bqk, scale=1.0)

        qk_stream(3)
        load_x8f(1)   # slot rotation WARs on qk3's reads

        # lk (stream 3) token-major via PE transpose
        lk_tok = p_lk.tile([128, TT, 128], BF, tag="lk", name="lk_tok")
        for tt in range(TT):
            pt = ps2.tile([128, 128], BF, tag="ps2", name="pt")
            nc.tensor.transpose(pt, qkT[:, 3, tt * 128:(tt + 1) * 128], ident)
            nc.vector.tensor_copy(lk_tok[:, tt, :], pt)

        # ---- kv chains + AllGather per e-half ----
        S_full = p_sf.tile([128, NG, HID], BF, tag="sf", name="S_full")

        def wsum_half(eh):
            e0 = eh * EH
            nc.sync.dma_start(S_full[:, 0, e0:e0 + EH], ap[f"rs_out{eh}"])
            for g in range(1, NG):
                nc.vector.tensor_add(S_full[:, g, e0:e0 + EH],
                                     S_full[:, g, e0:e0 + EH],
                                     S_full[:, 0, e0:e0 + EH])

        for eh in range(2):
            e0 = eh * EH
            t_half = p_so.tile([128, EH], BF, tag="so", name="t_half")
            for g in range(NG):
                pk = ps1.tile([128, 1024], F32, tag="ps1", name="pk")
                for jt in range(2):
                    for ec in range(2):
                        nc.tensor.matmul(
                            pk[:, ec * 512:(ec + 1) * 512],
                            lk_tok[:, 2 * g + jt, :],
                            v_h[:, 2 * g + jt, e0 + ec * 512:e0 + (ec + 1) * 512],
                            start=(jt == 0), stop=(jt == 1),
                            skip_group_check=True)
                dst = (S_full[:, g + 1, e0:e0 + EH] if g < NG - 1 else t_half)
                nc.scalar.activation(dst, pk, AF.Copy, bias=0.0, scale=1.0 / G)
            # exclusive-prefix over local groups on DVE (off the PE path)
            for g in range(2, NG):
                nc.vector.tensor_add(S_full[:, g, e0:e0 + EH],
                                     S_full[:, g, e0:e0 + EH],
                                     S_full[:, g - 1, e0:e0 + EH])
            nc.vector.tensor_add(t_half, t_half, S_full[:, NG - 1, e0:e0 + EH])
            # masked ReduceScatter computes the exclusive inter-core prefix
            # directly: rs_in block d = (my_rank < d) * t_half, so rank c
            # receives sum_{r<c} kv_r with no post-collective weighted sum.
            rs_in, rs_out = ap[f"rs_in{eh}"], ap[f"rs_out{eh}"]
            for dd in range(NC):
                for hh in range(2):
                    msk = p_so.tile([128, 512], BF, tag="msk", name="msk", bufs=2)
                    nc.vector.tensor_scalar_mul(
                        msk, t_half[:, hh * 512:(hh + 1) * 512],
                        wsumw[:, dd:dd + 1])
                    nc.gpsimd.dma_start(
                        rs_in[dd * 128:(dd + 1) * 128,
                              hh * 512:(hh + 1) * 512], msk)
            nc.gpsimd.collective_compute(
                "ReduceScatter", ALU.add, replica_groups=[list(range(NC))],
                ins=[rs_in], outs=[rs_out])

        for s in (2, 0, 1):
            qk_stream(s)

        # ---- sim/attn per group (conv band folded into bdiag/bcorn) ----
        attn0, attn1 = [], []
        for g in range(NG):
            i0 = g * G
            a0 = p_a0.tile([128, 256], BF, tag="a0")
            ps = ps2.tile([128, 256], F32, tag="ps2")
            nc.tensor.matmul(ps, qkT[:, 2, i0:i0 + 128], qkT[:, 0, i0:i0 + 256],
                             start=True, stop=True)
            nc.scalar.activation(a0, ps, AF.Relu, bias=0.0, scale=1.0 / G)
            nc.vector.tensor_mul(a0[:, 0:128], a0[:, 0:128], triu)
            nc.vector.tensor_mul(a0, a0, a0)
            nc.vector.tensor_add(a0[:, 0:128], a0[:, 0:128], bdiag)
            nc.vector.tensor_add(a0[:, 128:256], a0[:, 128:256], bcorn)
            attn0.append(a0)

            a1 = p_a1.tile([128, 256], BF, tag="a1")
            nc.vector.memset(a1[:, 0:128], 0.0)
            ps = ps2.tile([128, 256], F32, tag="ps2")
            nc.tensor.matmul(ps[:, 0:128], qkT[:, 2, i0 + 128:i0 + 256],
                             qkT[:, 0, i0 + 128:i0 + 256], start=True, stop=True)
            a1r = a1[:, 128:256]
            nc.scalar.activation(a1r, ps[:, 0:128], AF.Relu, bias=0.0, scale=1.0 / G)
            nc.vector.tensor_mul(a1r, a1r, triu)
            nc.vector.tensor_mul(a1r, a1r, a1r)
            nc.vector.tensor_add(a1r, a1r, bdiag)
            attn1.append(a1)


        wsum_half(0)
        wsum_half(1)

        # ---- quad + conv boundary -> outT ----
        outT = p_outT.tile([128, ET, T], BF, tag="outT")
        pdum = ps2.tile([128, 512], F32, tag="ps2", name="pdum")
        for eh in range(2):
            e0 = eh * EH
            for et in range(8):
                ec0 = e0 + et * 128
                po = ps1.tile([128, 1024], F32, tag="ps1", name="po")
                for g in range(NG):
                    c0 = g * G
                    if g % 2 == 0:
                        nc.tensor.matmul(pdum, ident, qkT[:, 0, 0:512],
                                         start=True, stop=True,
                                         skip_group_check=True)
                    nc.tensor.matmul(po[:, c0:c0 + 256],
                                     v_h[:, 2 * g, ec0:ec0 + 128], attn0[g],
                                     start=True, stop=False, skip_group_check=True)
                    nc.tensor.matmul(po[:, c0:c0 + 256],
                                     v_h[:, 2 * g + 1, ec0:ec0 + 128],
                                     attn1[g], start=False, stop=False,
                                     skip_group_check=True)
                    nc.tensor.matmul(po[:, c0:c0 + 256],
                                     tails[:, g, ec0:ec0 + 128], bprev,
                                     start=False, stop=True, skip_group_check=True)
                if et % 2 == 0:
                    nc.scalar.activation(outT[:, eh * 8 + et, :], po,
                                         AF.Copy, bias=0.0, scale=1.0)
                else:
                    nc.vector.tensor_copy(outT[:, eh * 8 + et, :], po)

        # gate weights (DMA while PE chews on quad)
        wgb = p_w2.tile([128, KB, HID], BF, tag="w2", name="wgb")
        for kt in range(KB):
            nc.scalar.dma_start(wgb[:, kt, :],
                                ap["wgb"][kt * 128:(kt + 1) * 128, :])
        wg8 = p_so.tile([128, KG8, HID], F8, tag="so", name="wg8")
        nc.scalar.dma_start(wg8, ap["wg8"].rearrange("(kt p) e -> p kt e", p=128))

        # scheduler fence: without it the scheduler hoists the lin matmuls
        # (which wait on the AllGather+wsum) ahead of quad and parks the PE
        tc.no_sync_barrier()

        # ---- lin joined via DVE add into outT ----
        for eh in range(2):
            for et in range(eh * 8, eh * 8 + 8):
                po = ps1.tile([128, 1024], F32, tag="ps1", name="po")
                for g in range(NG):
                    if g % 2 == 0:
                        nc.tensor.matmul(pdum, ident, qkT[:, 0, 0:512],
                                         start=True, stop=True,
                                         skip_group_check=True)
                    nc.tensor.matmul(po[:, g * G:(g + 1) * G],
                                     S_full[:, g, et * 128:(et + 1) * 128],
                                     qkT[:, 1, g * G:(g + 1) * G],
                                     start=True, stop=True, skip_group_check=True)
                nc.vector.tensor_add(outT[:, et, :], outT[:, et, :], po)

        # out-projection weights (DMA during lin/first gate)
        woutb = p_w1.tile([128, ET - KY8, DIM], BF, tag="w1", name="woutb")
        for kt in range(ET - KY8):
            nc.scalar.dma_start(woutb[:, kt, :],
                                ap["woutb"][kt * 128:(kt + 1) * 128, :])
        wout8 = p_lk.tile([128, KY8, DIM], F8, tag="lk", name="wout8")
        nc.scalar.dma_start(wout8, ap["wout8"].rearrange("(kt p) n -> p kt n",
                                                         p=128))

        if DEBUG_DUMPS:
            nc.sync.dma_start(ap["dbg_qkT"], qkT)
            nc.sync.dma_start(ap["dbg_vh"], v_h)
            nc.sync.dma_start(ap["dbg_outT"], outT)
            nc.sync.dma_start(ap["dbg_sfull"], S_full)

        # bf16 gate inputs (k-tiles 2..7), loaded during lin / earlier streams
        xtb = {}

        def load_xtb(s):
            halves = []
            for q in range(2):
                h = p_xtb.tile([128, KB // 2, T], BF, tag="xtb", name=f"xtb{s}_{q}")
                nc.sync.dma_start(
                    h, ap["xtb"][s, q * 384:(q + 1) * 384, :].rearrange(
                        "(kt p) t -> p kt t", p=128))
                halves.append(h)
            xtb[s] = halves

        load_xtb(0)
        load_xtb(1)

        # ---- gate + y per stream (th-paired gate, nch-paired y) ----
        for s in range(4):
            og8 = p_sf.tile([128, KY8, T], F8, tag="sf", name="og8")
            ogb = p_big.tile([128, ET - KY8, T], BF, tag="big", name="ogb")
            for et in range(ET):
                pg = ps1.tile([128, 1024], F32, tag="ps1", name="pg")
                for th in range(2):
                    nc.tensor.matmul(
                        pg[:, th * 512:(th + 1) * 512],
                        wg8[:, 0:KG8, et * 128:(et + 1) * 128],
                        x8f[s][:, 0:KG8, th * 512:(th + 1) * 512],
                        start=True, stop=False,
                        perf_mode=DRM, skip_group_check=True)
                for kt in range(KB):
                    xs_t = xtb[s][kt // 3][:, kt % 3, :]
                    for th in range(2):
                        nc.tensor.matmul(
                            pg[:, th * 512:(th + 1) * 512],
                            wgb[:, kt, et * 128:(et + 1) * 128],
                            xs_t[:, th * 512:(th + 1) * 512],
                            start=False, stop=(kt == KB - 1),
                            skip_group_check=True)
                if et < KY8:
                    gt = p_qog.tile([128, 1024], BF, tag="qog", name="gt")
                    nc.scalar.activation(gt, pg, AF.Silu,
                                         bias=bgate[:, et:et + 1], scale=1.0)
                    nc.vector.scalar_tensor_tensor(
                        og8[:, et, :], gt, XS, outT[:, et, :],
                        op0=ALU.mult, op1=ALU.mult)
                else:
                    eb = et - KY8
                    nc.scalar.activation(ogb[:, eb, :], pg, AF.Silu,
                                         bias=bgate[:, et:et + 1], scale=1.0)
                    nc.vector.tensor_mul(ogb[:, eb, :], ogb[:, eb, :],
                                         outT[:, et, :])
            if s < 2:
                load_xtb(s + 2)
            if s == 0:
                load_x8f(1)
            elif s == 1:
                load_x8f(2)
            elif s == 2:
                load_x8f(3)
            for tl in range(TT):
                tsl = slice(tl * 128, (tl + 1) * 128)
                pn = ps1.tile([128, 1024], F32, tag="ps1", name="pn")
                for kp in range(KY8 // 2):
                    for nch in range(2):
                        nc.tensor.matmul(
                            pn[:, nch * 512:(nch + 1) * 512],
                            og8[:, 2 * kp:2 * kp + 2, tsl],
                            wout8[:, 2 * kp:2 * kp + 2,
                                  nch * 512:(nch + 1) * 512],
                            start=(kp == 0), stop=False,
                            perf_mode=DRM, skip_group_check=True)
                for kt in range(ET - KY8):
                    for nch in range(2):
                        nc.tensor.matmul(
                            pn[:, nch * 512:(nch + 1) * 512],
                            ogb[:, kt, tsl],
                            woutb[:, kt, nch * 512:(nch + 1) * 512],
                            start=False,
                            stop=(kt == ET - KY8 - 1 and not WITH_OBIAS),
                            skip_group_check=True)
                if WITH_OBIAS:
                    for nch in range(2):
                        nc.tensor.matmul(pn[:, nch * 512:(nch + 1) * 512],
                                         ones_t[0:1, 0:128],
                                         bout[0:1, nch * 512:(nch + 1) * 512],
                                         start=False, stop=True,
                                         skip_group_check=True)
                xr = p_sob.tile([128, 1024], F32, tag="xr", name="xr")
                nc.sync.dma_start(xr, ap["xtok"][s, tsl, :])
                ysb = p_x08.tile([128, 1024], F32, tag="x08", name="ysb")
                nc.vector.scalar_tensor_tensor(
                    ysb, pn, 0.0, xr, op0=ALU.add, op1=ALU.add)
                nc.sync.dma_start(ap["y"][s, tsl, :], ysb)


def build_nc(with_vbias=None, with_obias=None):
    global WITH_VBIAS, WITH_OBIAS
    if with_vbias is not None:
        WITH_VBIAS = with_vbias
    if with_obias is not None:
        WITH_OBIAS = with_obias
    nc = bacc.Bacc("TRN2", target_bir_lowering=False, debug=False, num_devices=NC)
    ap = {}

    def dram(name, shape, dt, kind=None, addr_space=None):
        kw = {}
        if kind:
            kw["kind"] = kind
        if addr_space:
            kw["addr_space"] = addr_space
        ap[name] = nc.dram_tensor(name, shape, dt, **kw).ap()

    dram("xtb", [4, KB * 128, T], BF, kind="ExternalInput")
    dram("xt8", [3, DIM, T], F8, kind="ExternalInput")
    dram("xt08", [DIM, T], F8, kind="ExternalInput")
    dram("xh8", [DIM, 32], F8, kind="ExternalInput")
    dram("xtok", [4, T, DIM], F32, kind="ExternalInput")
    dram("wv8", [DIM, HID], F8, kind="ExternalInput")
    dram("wgb", [KB * 128, HID], BF, kind="ExternalInput")
    dram("wg8", [KG8 * 128, HID], F8, kind="ExternalInput")
    dram("wqk8", [DIM, DQK], F8, kind="ExternalInput")
    dram("woutb", [(ET - KY8) * 128, DIM], BF, kind="ExternalInput")
    dram("wout8", [KY8 * 128, DIM], F8, kind="ExternalInput")
    dram("wvb", [1, HID], BF, kind="ExternalInput")
    dram("bout", [1, DIM], BF, kind="ExternalInput")
    dram("bgate", [128, ET], F32, kind="ExternalInput")
    dram("bqk", [128, 1], F32, kind="ExternalInput")
    dram("triu", [128, 128], BF, kind="ExternalInput")
    dram("bdiag", [128, 128], BF, kind="ExternalInput")
    dram("bcorn", [128, 128], BF, kind="ExternalInput")
    dram("bprev", [32, 256], BF, kind="ExternalInput")
    dram("hmask", [32, 1], F32, kind="ExternalInput")
    if DEBUG_DUMPS:
        dram("dbg_qkT", [128, 4, T], BF, kind="ExternalOutput")
        dram("dbg_vh", [128, TT, HID], F8, kind="ExternalOutput")
        dram("dbg_outT", [128, ET, T], BF, kind="ExternalOutput")
        dram("dbg_sfull", [128, NG, HID], BF, kind="ExternalOutput")
    dram("cc_warm_in", [128, 16], BF)
    dram("cc_warm_out", [NC * 128, 16], BF, addr_space="Shared")
    dram("wsumw", [128, NC], F32, kind="ExternalInput")
    dram("rs_in0", [NC * 128, EH], BF)
    dram("rs_out0", [128, EH], BF)
    dram("rs_in1", [NC * 128, EH], BF)
    dram("rs_out1", [128, EH], BF)
    dram("y", [4, T, DIM], F32, kind="ExternalOutput")

    with tile.TileContext(nc) as tc:
        _emit(tc, ap)
    nc.compile()
    return nc


def _f8(a):
    return np.clip(a, -240.0, 240.0).astype(fp8)


def host_prep(inputs):
    """Pure layout transforms: shard, transpose, cast, build conv-band consts."""
    x = np.ascontiguousarray(np.asarray(inputs["x"], np.float32)[0])  # [4, N, DIM]
    W_h = np.asarray(inputs["W_h"], np.float32)
    b_h = np.asarray(inputs["b_h"], np.float32)
    W_qk = np.asarray(inputs["W_qk"], np.float32)
    b_qk = np.asarray(inputs["b_qk"], np.float32)
    W_out = np.asarray(inputs["W_out"], np.float32)
    b_out = np.asarray(inputs["b_out"], np.float32)
    cw = np.asarray(inputs["conv_w"], np.float32)

    jj = np.arange(128)[:, None]
    ii = np.arange(128)[None, :]
    d = ii - jj
    triu = (ii >= jj).astype(bf16)
    bdiag = np.where((d >= 0) & (d <= 31), cw[np.clip(31 - d, 0, 62)], 0.0).astype(bf16)
    dc = (ii + 128) - jj
    bcorn = np.where((dc >= 0) & (dc <= 31),
                     cw[np.clip(31 - dc, 0, 62)], 0.0).astype(bf16)
    jt = np.arange(32)[:, None]
    ip = np.arange(32)[None, :]
    dp = ip + 32 - jt
    bprev = np.zeros((32, 256), np.float32)
    bprev[:, :32] = np.where((dp >= 1) & (dp <= 31),
                             cw[np.clip(31 - dp, 0, 62)], 0.0)
    bprev = bprev.astype(bf16)

    kb0 = KG8 * 128
    ky0 = KY8 * 128
    common = {
        "wv8": _f8(W_h[:, :HID] * WS),
        "wgb": np.ascontiguousarray(W_h[kb0:, HID:]).astype(bf16),
        "wg8": _f8(W_h[:kb0, HID:] * WS),
        "wqk8": _f8(W_qk * WS),
        "woutb": np.ascontiguousarray(W_out[ky0:, :]).astype(bf16),
        "wout8": _f8(W_out[:ky0, :] * WS),
        "wvb": b_h[None, :HID].astype(bf16),
        "bout": b_out[None, :].astype(bf16),
        "bgate": np.ascontiguousarray(b_h[HID:].reshape(ET, 128).T).astype(np.float32),
        "bqk": b_qk[:, None].astype(np.float32),
        "triu": triu, "bdiag": bdiag, "bcorn": bcorn, "bprev": bprev,
    }

    in_maps = []
    for c in range(NC):
        sl = slice(c * T, (c + 1) * T)
        x_c = x[:, sl, :]
        xtb = np.zeros((4, KB * 128, T), bf16)
        xt8 = np.zeros((3, DIM, T), fp8)
        for s in range(4):
            xT = x_c[s].T
            xtb[s] = xT[kb0:].astype(bf16)
            if s > 0:
                xt8[s - 1] = _f8(xT * XS)
        xt08 = _f8(x_c[0].T * XS)
        if c > 0:
            xh8 = _f8(np.ascontiguousarray(x[0, c * T - 32:c * T, :].T) * XS)
        else:
            xh8 = np.zeros((DIM, 32), fp8)
        m = dict(common)
        m["xtb"] = xtb
        m["xt8"] = xt8
        m["xt08"] = xt08
        m["xh8"] = xh8
        m["xtok"] = np.ascontiguousarray(x_c)
        m["hmask"] = np.full((32, 1), 1.0 if c > 0 else 0.0, np.float32)
        w = np.zeros((128, NC), np.float32)
        w[:, c + 1:] = 1.0
        m["wsumw"] = w
        in_maps.append(m)
    return in_maps


_NC_PROG = None
_NC_FLAGS = None


def kernel(**inputs):
    global _NC_PROG, _NC_FLAGS
    b_h = np.asarray(inputs["b_h"], np.float32)
    b_out = np.asarray(inputs["b_out"], np.float32)
    flags = (bool(np.any(b_h[:HID])), bool(np.any(b_out)))
    if _NC_PROG is None or _NC_FLAGS != flags:
        _NC_PROG = build_nc(with_vbias=flags[0], with_obias=flags[1])
        _NC_FLAGS = flags
    in_maps = host_prep(inputs)
    res = run_bass_kernel_spmd(_NC_PROG, in_maps, list(range(NC)))
    y = np.stack([res.results[c]["y"] for c in range(NC)], axis=1)  # [4, NC, T, DIM]
    return np.ascontiguousarray(y.reshape(4, NSEQ, DIM)[None]).astype(np.float32)


# revision 25
# speedup vs baseline: 1.0937x; 1.0937x over previous
"""Trainium2 Bass kernel for nn_FLASH_40458591928592 (sparse_attention).

Sequence-sharded over 8 NeuronCores: 1024 tokens (= 4 groups of 256) per core.
Mixed precision, validated against a numpy e4m3 simulation (rel 1.66e-2 < 2e-2):
  qk GEMM : fully fp8 DoubleRow (x*0.25 stationary-free scales, W*4)
  v GEMM  : fully fp8 DoubleRow; v_h and tails stored fp8 (storage only)
  gate    : k-tiles 0-1 of 8 fp8 DR, rest bf16 (same PSUM, products at scale 1)
  y       : HID k-tiles 0-3 of 16 fp8 DR (og et 0-3 written fp8*0.25 by DVE)

Phase order keeps the PE dense and the HAM clock warm:
  dummy warmup matmuls through the DMA lead-in -> v -> qk(3,2,0,1) -> sim/attn
  -> lk transposes -> kv chains (AllGather per e-half fires ~60us) -> quad+conv
  (overlapping the collectives) -> lin -> gate+y per stream.
One LDWEIGHTS feeds 2-4 matmuls everywhere (ec/ch/th/nch pairing).
SBUF is tag-chained across serial phases (vh->ogb, wv8->woutb, qkT->gt,
S_full->og8, t_half->wg8, lk->wout8, S_offb->xr, tails->ysb).
"""

from contextlib import ExitStack

import numpy as np
import ml_dtypes

import concourse.tile as tile
from concourse import bacc, mybir
from concourse.bass_utils import run_bass_kernel_spmd
from concourse.masks import make_identity

BF = mybir.dt.bfloat16
F8 = mybir.dt.float8e4
F32 = mybir.dt.float32
bf16 = ml_dtypes.bfloat16
fp8 = ml_dtypes.float8_e4m3
DRM = mybir.MatmulPerfMode.DoubleRow

G = 256
DIM = 1024
HID = 2048
DQK = 128
NSEQ = 8192
NC = 8
T = NSEQ // NC        # 1024 tokens per core
NG = T // G           # 4 groups per core
KD = DIM // 128       # 8 k-tiles over dim
ET = HID // 128       # 16 e-tiles over hid
TT = T // 128         # 8 token tiles
EH = HID // 2         # 1024 cols per e-half

KG8 = 2               # gate fp8 k-tiles (of KD); one DR pair
KY8 = 4               # y fp8 k-tiles (of ET); must be even
KB = KD - KG8         # bf16 k-tiles for gate
XS = 0.25             # fp8 x-side scale
WS = 4.0              # fp8 w-side scale (XS*WS == 1 -> shared-PSUM)
NWARM = 18            # HAM warmup dummy matmuls

AF = mybir.ActivationFunctionType
ALU = mybir.AluOpType

DEBUG_DUMPS = False
WITH_VBIAS = True
WITH_OBIAS = True


def _emit(tc, ap):
    nc = tc.nc
    with ExitStack() as ctx:
        pass

        consts = ctx.enter_context(tc.tile_pool(name="consts", bufs=1))
        p_x8 = ctx.enter_context(tc.tile_pool(name="x8", bufs=2))
        p_xtb = ctx.enter_context(tc.tile_pool(name="xtb", bufs=3))
        p_x08 = ctx.enter_context(tc.tile_pool(name="x08", bufs=1))
        p_big = ctx.enter_context(tc.tile_pool(name="big", bufs=1))   # vh8 -> ogb
        p_qog = ctx.enter_context(tc.tile_pool(name="qog", bufs=1))   # qkT -> gt
        p_lk = ctx.enter_context(tc.tile_pool(name="lk", bufs=1))     # lk -> wout8
        p_w1 = ctx.enter_context(tc.tile_pool(name="w1", bufs=1))     # wv8 -> woutb
        p_w2 = ctx.enter_context(tc.tile_pool(name="w2", bufs=1))     # wgb
        p_tails = ctx.enter_context(tc.tile_pool(name="tails", bufs=1))  # -> ysb
        p_so = ctx.enter_context(tc.tile_pool(name="so", bufs=2))     # t_half -> wg8
        p_sob = ctx.enter_context(tc.tile_pool(name="sob", bufs=2))   # xr
        p_sf = ctx.enter_context(tc.tile_pool(name="sf", bufs=1))     # S_full -> og8
        p_tr = ctx.enter_context(tc.tile_pool(name="tr", bufs=4))
        p_a0 = ctx.enter_context(tc.tile_pool(name="a0", bufs=4))
        p_a1 = ctx.enter_context(tc.tile_pool(name="a1", bufs=4))
        p_outT = ctx.enter_context(tc.tile_pool(name="outT", bufs=1))
        ps1 = ctx.enter_context(tc.tile_pool(name="ps1", bufs=3, space="PSUM"))
        ps2 = ctx.enter_context(tc.tile_pool(name="ps2", bufs=2, space="PSUM"))

        # warm-up collective first: its ~40us post-trigger setup runs in the
        # shadow of the input DMAs, so the real AllGathers start immediately
        cwarm = consts.tile([128, 16], BF, tag="cwarm")
        nc.vector.memset(cwarm, 0.0)
        nc.sync.dma_start(ap["cc_warm_in"], cwarm)
        nc.gpsimd.collective_compute(
            "AllGather", ALU.bypass, replica_groups=[list(range(NC))],
            ins=[ap["cc_warm_in"]], outs=[ap["cc_warm_out"]])

        # ---- HAM warmup: keep PE busy through the DMA lead-in ----
        ident = consts.tile([128, 128], BF, tag="ident")
        make_identity(nc, ident)
        for _ in range(NWARM):
            pw = ps2.tile([128, 128], F32, tag="ps2", name="pw")
            nc.tensor.matmul(pw, ident, ident, start=True, stop=True)

        # ---- first DMAs: v-GEMM inputs, then qk weights ----
        x08 = p_x08.tile([128, KD, T], F8, tag="x08")
        wv8 = p_w1.tile([128, KD, HID], F8, tag="w1")
        for q in range(4):
            nc.sync.dma_start(
                x08[:, q * 2:(q + 1) * 2, :],
                ap["xt08"][q * 256:(q + 1) * 256, :].rearrange(
                    "(kt p) t -> p kt t", p=128))
            eng = nc.scalar if q < 2 else nc.gpsimd
            eng.dma_start(
                wv8[:, q * 2:(q + 1) * 2, :],
                ap["wv8"][q * 256:(q + 1) * 256, :].rearrange(
                    "(kt p) e -> p kt e", p=128))
        bqk = consts.tile([128, 1], F32, tag="bqk")
        nc.scalar.dma_start(bqk, ap["bqk"])
        wqk8 = consts.tile([128, KD, DQK], F8, tag="wqk8")
        nc.scalar.dma_start(wqk8, ap["wqk8"].rearrange("(kt p) q -> p kt q", p=128))
        xh8 = consts.tile([128, KD, 32], F8, tag="xh8")
        nc.scalar.dma_start(xh8, ap["xh8"].rearrange("(kt p) t -> p kt t", p=128))

        x8f = {0: x08}

        def load_x8f(s):
            t8 = p_x8.tile([128, KD, T], F8, tag="x8", name=f"x8_{s}")
            nc.sync.dma_start(t8, ap["xt8"][s - 1].rearrange("(kt p) t -> p kt t",
                                                             p=128))
            x8f[s] = t8

        load_x8f(3)
        load_x8f(2)

        # remaining consts (DMA behind weights on scalar queue)
        triu = consts.tile([128, 128], BF, tag="triu")
        nc.scalar.dma_start(triu, ap["triu"])
        bdiag = consts.tile([128, 128], BF, tag="bdiag")
        nc.scalar.dma_start(bdiag, ap["bdiag"])
        bcorn = consts.tile([128, 128], BF, tag="bcorn")
        nc.scalar.dma_start(bcorn, ap["bcorn"])
        bprev = consts.tile([32, 256], BF, tag="bprev")
        nc.scalar.dma_start(bprev, ap["bprev"])
        hmask = consts.tile([32, 1], F32, tag="hmask")
        nc.scalar.dma_start(hmask, ap["hmask"])
        wsumw = consts.tile([128, NC], F32, tag="wsumw")
        nc.scalar.dma_start(wsumw, ap["wsumw"])
        bgate = consts.tile([128, ET], F32, tag="bgate")
        nc.scalar.dma_start(bgate, ap["bgate"])
        if WITH_VBIAS or WITH_OBIAS:
            ones_t = consts.tile([1, 1024], BF, tag="ones")
            nc.vector.memset(ones_t, 1.0)
        if WITH_VBIAS:
            wvb = consts.tile([1, HID], BF, tag="wvb")
            nc.scalar.dma_start(wvb, ap["wvb"])
        if WITH_OBIAS:
            bout = consts.tile([1, DIM], BF, tag="bout")
            nc.scalar.dma_start(bout, ap["bout"])

        # ---- v GEMM: fp8 DR, one xt-pair LDWEIGHTS feeds 4 e-chunks ----
        v_h = p_big.tile([128, TT, HID], F8, tag="big", name="v_h")
        for tt in range(TT):
            pv = [ps1.tile([128, 1024], F32, tag="ps1", name="pv")
                  for _ in range(2)]
            for kp in range(KD // 2):
                for ec in range(4):
                    nc.tensor.matmul(
                        pv[ec // 2][:, (ec % 2) * 512:(ec % 2 + 1) * 512],
                        x08[:, 2 * kp:2 * kp + 2, tt * 128:(tt + 1) * 128],
                        wv8[:, 2 * kp:2 * kp + 2, ec * 512:(ec + 1) * 512],
                        start=(kp == 0),
                        stop=(kp == KD // 2 - 1 and not WITH_VBIAS),
                        perf_mode=DRM, skip_group_check=True)
            if WITH_VBIAS:
                for ec in range(4):
                    nc.tensor.matmul(pv[ec // 2][:, (ec % 2) * 512:(ec % 2 + 1) * 512],
                                     ones_t[0:1, 0:128],
                                     wvb[0:1, ec * 512:(ec + 1) * 512],
                                     start=False, stop=True, skip_group_check=True)
            for eh in range(2):
                nc.scalar.activation(v_h[:, tt, eh * 1024:(eh + 1) * 1024], pv[eh],
                                     AF.Silu, bias=0.0, scale=1.0)

        # halo: last 32 tokens of the previous core (masked on core 0)
        tails = p_tails.tile([32, NG, HID], F8, tag="tails")
        ph = [ps1.tile([32, 1024], F32, tag="ps1", name="ph") for _ in range(2)]
        for kp in range(KD // 2):
            for ec in range(4):
                nc.tensor.matmul(
                    ph[ec // 2][:, (ec % 2) * 512:(ec % 2 + 1) * 512],
                    xh8[:, 2 * kp:2 * kp + 2, :],
                    wv8[:, 2 * kp:2 * kp + 2, ec * 512:(ec + 1) * 512],
                    start=(kp == 0), stop=(kp == KD // 2 - 1 and not WITH_VBIAS),
                    perf_mode=DRM, skip_group_check=True)
        if WITH_VBIAS:
            for ec in range(4):
                nc.tensor.matmul(ph[ec // 2][:, (ec % 2) * 512:(ec % 2 + 1) * 512],
                                 ones_t[0:1, 0:32],
                                 wvb[0:1, ec * 512:(ec + 1) * 512],
                                 start=False, stop=True, skip_group_check=True)
        for eh in range(2):
            nc.scalar.activation(tails[:, 0, eh * 1024:(eh + 1) * 1024], ph[eh],
                                 AF.Silu, bias=0.0, scale=1.0)
            nc.vector.tensor_scalar_mul(tails[:, 0, eh * 1024:(eh + 1) * 1024],
                                        tails[:, 0, eh * 1024:(eh + 1) * 1024],
                                        hmask)
        for g in range(1, NG):
            nc.scalar.dma_start(tails[:, g, :], v_h[96:128, 2 * g - 1, :])

        # ---- qk streams: fully fp8 DR, ch-paired ----
        qkT = p_qog.tile([128, 4, T], BF, tag="qog", name="qkT")

        def qk_stream(s):
            pc = ps1.tile([128, 1024], F32, tag="ps1", name="pc")
            for kp in range(KD // 2):
                for ch in range(2):
                    nc.tensor.matmul(pc[:, ch * 512:(ch + 1) * 512],
                                     wqk8[:, 2 * kp:2 * kp + 2, :],
                                     x8f[s][:, 2 * kp:2 * kp + 2,
                                            ch * 512:(ch + 1) * 512],
                                     start=(kp == 0), stop=(kp == KD // 2 - 1),
                                     perf_mode=DRM, skip_group_check=True)
            nc.scalar.activation(qkT[:, s, :], pc, AF.Silu, bias=bqk, scale=1.0)

        qk_stream(3)
        load_x8f(1)   # slot rotation WARs on qk3's reads

        # lk (stream 3) token-major via PE transpose
        lk_tok = p_lk.tile([128, TT, 128], BF, tag="lk", name="lk_tok")
        for tt in range(TT):
            pt = ps2.tile([128, 128], BF, tag="ps2", name="pt")
            nc.tensor.transpose(pt, qkT[:, 3, tt * 128:(tt + 1) * 128], ident)
            nc.vector.tensor_copy(lk_tok[:, tt, :], pt)

        # ---- kv chains + AllGather per e-half ----
        S_full = p_sf.tile([128, NG, HID], BF, tag="sf", name="S_full")

        def wsum_half(eh):
            e0 = eh * EH
            cc_out = ap[f"cc_out{eh}"]
            for r in range(NC):
                for hh in range(2):
                    s0 = S_full[:, 0, e0 + hh * 512:e0 + (hh + 1) * 512]
                    tr = p_tr.tile([128, 512], F8, tag="tr")
                    nc.sync.dma_start(
                        tr, cc_out[r * 128:(r + 1) * 128,
                                   hh * 512:(hh + 1) * 512])
                    if r == 0:
                        nc.vector.tensor_scalar_mul(s0, tr, wsumw[:, 0:1])
                    else:
                        nc.vector.scalar_tensor_tensor(
                            s0, tr, wsumw[:, r:r + 1], s0,
                            op0=ALU.mult, op1=ALU.add)
            for g in range(1, NG):
                nc.vector.tensor_add(S_full[:, g, e0:e0 + EH],
                                     S_full[:, g, e0:e0 + EH],
                                     S_full[:, 0, e0:e0 + EH])

        for eh in range(2):
            e0 = eh * EH
            t_half = p_so.tile([128, EH], F8, tag="so", name="t_half")
            for g in range(NG):
                pk = ps1.tile([128, 1024], F32, tag="ps1", name="pk")
                for jt in range(2):
                    for ec in range(2):
                        nc.tensor.matmul(
                            pk[:, ec * 512:(ec + 1) * 512],
                            lk_tok[:, 2 * g + jt, :],
                            v_h[:, 2 * g + jt, e0 + ec * 512:e0 + (ec + 1) * 512],
                            start=(jt == 0), stop=(jt == 1),
                            skip_group_check=True)
                dst = (S_full[:, g + 1, e0:e0 + EH] if g < NG - 1 else t_half)
                nc.scalar.activation(dst, pk, AF.Copy, bias=0.0, scale=1.0 / G)
            # exclusive-prefix over local groups on DVE (off the PE path)
            for g in range(2, NG):
                nc.vector.tensor_add(S_full[:, g, e0:e0 + EH],
                                     S_full[:, g, e0:e0 + EH],
                                     S_full[:, g - 1, e0:e0 + EH])
            nc.vector.tensor_add(t_half, t_half, S_full[:, NG - 1, e0:e0 + EH])
            # fire the AllGather; the weighted sum is emitted after attn so
            # the DVE stream is not blocked waiting on the collective
            cc_in, cc_out = ap[f"cc_in{eh}"], ap[f"cc_out{eh}"]
            nc.scalar.dma_start(cc_in, t_half)
            nc.gpsimd.collective_compute(
                "AllGather", ALU.bypass, replica_groups=[list(range(NC))],
                ins=[cc_in], outs=[cc_out])

        for s in (2, 0, 1):
            qk_stream(s)

        # ---- sim/attn per group (conv band folded into bdiag/bcorn) ----
        attn0, attn1 = [], []
        for g in range(NG):
            i0 = g * G
            a0 = p_a0.tile([128, 256], BF, tag="a0")
            ps = ps2.tile([128, 256], F32, tag="ps2")
            nc.tensor.matmul(ps, qkT[:, 2, i0:i0 + 128], qkT[:, 0, i0:i0 + 256],
                             start=True, stop=True)
            nc.scalar.activation(a0, ps, AF.Relu, bias=0.0, scale=1.0 / G)
            nc.vector.tensor_mul(a0[:, 0:128], a0[:, 0:128], triu)
            nc.vector.tensor_mul(a0, a0, a0)
            nc.vector.tensor_add(a0[:, 0:128], a0[:, 0:128], bdiag)
            nc.vector.tensor_add(a0[:, 128:256], a0[:, 128:256], bcorn)
            attn0.append(a0)

            a1 = p_a1.tile([128, 256], BF, tag="a1")
            nc.vector.memset(a1[:, 0:128], 0.0)
            ps = ps2.tile([128, 256], F32, tag="ps2")
            nc.tensor.matmul(ps[:, 0:128], qkT[:, 2, i0 + 128:i0 + 256],
                             qkT[:, 0, i0 + 128:i0 + 256], start=True, stop=True)
            a1r = a1[:, 128:256]
            nc.scalar.activation(a1r, ps[:, 0:128], AF.Relu, bias=0.0, scale=1.0 / G)
            nc.vector.tensor_mul(a1r, a1r, triu)
            nc.vector.tensor_mul(a1r, a1r, a1r)
            nc.vector.tensor_add(a1r, a1r, bdiag)
            attn1.append(a1)


        wsum_half(0)
        wsum_half(1)

        # ---- quad + conv boundary -> outT ----
        outT = p_outT.tile([128, ET, T], BF, tag="outT")
        pdum = ps2.tile([128, 512], F32, tag="ps2", name="pdum")
        for eh in range(2):
            e0 = eh * EH
            for et in range(8):
                ec0 = e0 + et * 128
                po = ps1.tile([128, 1024], F32, tag="ps1", name="po")
                for g in range(NG):
                    c0 = g * G
                    if g % 2 == 0:
                        nc.tensor.matmul(pdum, ident, qkT[:, 0, 0:512],
                                         start=True, stop=True,
                                         skip_group_check=True)
                    nc.tensor.matmul(po[:, c0:c0 + 256],
                                     v_h[:, 2 * g, ec0:ec0 + 128], attn0[g],
                                     start=True, stop=False, skip_group_check=True)
                    nc.tensor.matmul(po[:, c0:c0 + 256],
                                     v_h[:, 2 * g + 1, ec0:ec0 + 128],
                                     attn1[g], start=False, stop=False,
                                     skip_group_check=True)
                    nc.tensor.matmul(po[:, c0:c0 + 256],
                                     tails[:, g, ec0:ec0 + 128], bprev,
                                     start=False, stop=True, skip_group_check=True)
                if et % 2 == 0:
                    nc.scalar.activation(outT[:, eh * 8 + et, :], po,
                                         AF.Copy, bias=0.0, scale=1.0)
                else:
                    nc.vector.tensor_copy(outT[:, eh * 8 + et, :], po)

        # gate weights (DMA while PE chews on quad)
        wgb = p_w2.tile([128, KB, HID], BF, tag="w2", name="wgb")
        for kt in range(KB):
            nc.scalar.dma_start(wgb[:, kt, :],
                                ap["wgb"][kt * 128:(kt + 1) * 128, :])
        wg8 = p_so.tile([128, KG8, HID], F8, tag="so", name="wg8")
        nc.scalar.dma_start(wg8, ap["wg8"].rearrange("(kt p) e -> p kt e", p=128))

        # scheduler fence: without it the scheduler hoists the lin matmuls
        # (which wait on the AllGather+wsum) ahead of quad and parks the PE
        tc.no_sync_barrier()

        # ---- lin joined via DVE add into outT ----
        for eh in range(2):
            for et in range(eh * 8, eh * 8 + 8):
                po = ps1.tile([128, 1024], F32, tag="ps1", name="po")
                for g in range(NG):
                    if g % 2 == 0:
                        nc.tensor.matmul(pdum, ident, qkT[:, 0, 0:512],
                                         start=True, stop=True,
                                         skip_group_check=True)
                    nc.tensor.matmul(po[:, g * G:(g + 1) * G],
                                     S_full[:, g, et * 128:(et + 1) * 128],
                                     qkT[:, 1, g * G:(g + 1) * G],
                                     start=True, stop=True, skip_group_check=True)
                nc.vector.tensor_add(outT[:, et, :], outT[:, et, :], po)

        # out-projection weights (DMA during lin/first gate)
        woutb = p_w1.tile([128, ET - KY8, DIM], BF, tag="w1", name="woutb")
        for kt in range(ET - KY8):
            nc.scalar.dma_start(woutb[:, kt, :],
                                ap["woutb"][kt * 128:(kt + 1) * 128, :])
        wout8 = p_lk.tile([128, KY8, DIM], F8, tag="lk", name="wout8")
        nc.scalar.dma_start(wout8, ap["wout8"].rearrange("(kt p) n -> p kt n",
                                                         p=128))

        if DEBUG_DUMPS:
            nc.sync.dma_start(ap["dbg_qkT"], qkT)
            nc.sync.dma_start(ap["dbg_vh"], v_h)
            nc.sync.dma_start(ap["dbg_outT"], outT)
            nc.sync.dma_start(ap["dbg_sfull"], S_full)

        # bf16 gate inputs (k-tiles 2..7), loaded during lin / earlier streams
        xtb = {}

        def load_xtb(s):
            halves = []
            for q in range(2):
                h = p_xtb.tile([128, KB // 2, T], BF, tag="xtb", name=f"xtb{s}_{q}")
                nc.sync.dma_start(
                    h, ap["xtb"][s, q * 384:(q + 1) * 384, :].rearrange(
                        "(kt p) t -> p kt t", p=128))
                halves.append(h)
            xtb[s] = halves

        load_xtb(0)
        load_xtb(1)

        # ---- gate + y per stream (th-paired gate, nch-paired y) ----
        for s in range(4):
            og8 = p_sf.tile([128, KY8, T], F8, tag="sf", name="og8")
            ogb = p_big.tile([128, ET - KY8, T], BF, tag="big", name="ogb")
            for et in range(ET):
                pg = ps1.tile([128, 1024], F32, tag="ps1", name="pg")
                for th in range(2):
                    nc.tensor.matmul(
                        pg[:, th * 512:(th + 1) * 512],
                        wg8[:, 0:KG8, et * 128:(et + 1) * 128],
                        x8f[s][:, 0:KG8, th * 512:(th + 1) * 512],
                        start=True, stop=False,
                        perf_mode=DRM, skip_group_check=True)
                for kt in range(KB):
                    xs_t = xtb[s][kt // 3][:, kt % 3, :]
                    for th in range(2):
                        nc.tensor.matmul(
                            pg[:, th * 512:(th + 1) * 512],
                            wgb[:, kt, et * 128:(et + 1) * 128],
                            xs_t[:, th * 512:(th + 1) * 512],
                            start=False, stop=(kt == KB - 1),
                            skip_group_check=True)
                if et < KY8:
                    gt = p_qog.tile([128, 1024], BF, tag="qog", name="gt")
                    nc.scalar.activation(gt, pg, AF.Silu,
                                         bias=bgate[:, et:et + 1], scale=1.0)
                    nc.vector.scalar_tensor_tensor(
                        og8[:, et, :], gt, XS, outT[:, et, :],
                        op0=ALU.mult, op1=ALU.mult)
                else:
                    eb = et - KY8
                    nc.scalar.activation(ogb[:, eb, :], pg, AF.Silu,
                                         bias=bgate[:, et:et + 1], scale=1.0)
                    nc.vector.tensor_mul(ogb[:, eb, :], ogb[:, eb, :],
                                         outT[:, et, :])
            if s < 2:
                load_xtb(s + 2)
            if s == 0:
                load_x8f(1)
            elif s == 1:
                load_x8f(2)
            elif s == 2:
                load_x8f(3)
            for tl in range(TT):
                tsl = slice(tl * 128, (tl + 1) * 128)
                pn = ps1.tile([128, 1024], F32, tag="ps1", name="pn")
                for kp in range(KY8 // 2):
                    for nch in range(2):
                        nc.tensor.matmul(
                            pn[:, nch * 512:(nch + 1) * 512],
                            og8[:, 2 * kp:2 * kp + 2, tsl],
                            wout8[:, 2 * kp:2 * kp + 2,
                                  nch * 512:(nch + 1) * 512],
                            start=(kp == 0), stop=False,
                            perf_mode=DRM, skip_group_check=True)
                for kt in range(ET - KY8):
                    for nch in range(2):
                        nc.tensor.matmul(
                            pn[:, nch * 512:(nch + 1) * 512],
                            ogb[:, kt, tsl],
                            woutb[:, kt, nch * 512:(nch + 1) * 512],
                            start=False,
                            stop=(kt == ET - KY8 - 1 and not WITH_OBIAS),
                            skip_group_check=True)
                if WITH_OBIAS:
                    for nch in range(2):
                        nc.tensor.matmul(pn[:, nch * 512:(nch + 1) * 512],
                                         ones_t[0:1, 0:128],
                                         bout[0:1, nch * 512:(nch + 1) * 512],
                                         start=False, stop=True,
                                         skip_group_check=True)
                xr = p_sob.tile([128, 1024], F32, tag="xr", name="xr")
                nc.sync.dma_start(xr, ap["xtok"][s, tsl, :])
                ysb = p_x08.tile([128, 1024], F32, tag="x08", name="ysb")
                nc.vector.scalar_tensor_tensor(
                    ysb, pn, 0.0, xr, op0=ALU.add, op1=ALU.add)
                nc.sync.dma_start(ap["y"][s, tsl, :], ysb)


def build_nc(with_vbias=None, with_obias=None):
    global WITH_VBIAS, WITH_OBIAS
    if with_vbias is not None:
        WITH_VBIAS = with_vbias
    if with_obias is not None:
        WITH_OBIAS = with_obias
    nc = bacc.Bacc("TRN2", target_bir_lowering=False, debug=False, num_devices=NC)
    ap = {}

    def dram(name, shape, dt, kind=None, addr_space=None):
        kw = {}
        if kind:
            kw["kind"] = kind
        if addr_space:
            kw["addr_space"] = addr_space
        ap[name] = nc.dram_tensor(name, shape, dt, **kw).ap()

    dram("xtb", [4, KB * 128, T], BF, kind="ExternalInput")
    dram("xt8", [3, DIM, T], F8, kind="ExternalInput")
    dram("xt08", [DIM, T], F8, kind="ExternalInput")
    dram("xh8", [DIM, 32], F8, kind="ExternalInput")
    dram("xtok", [4, T, DIM], F32, kind="ExternalInput")
    dram("wv8", [DIM, HID], F8, kind="ExternalInput")
    dram("wgb", [KB * 128, HID], BF, kind="ExternalInput")
    dram("wg8", [KG8 * 128, HID], F8, kind="ExternalInput")
    dram("wqk8", [DIM, DQK], F8, kind="ExternalInput")
    dram("woutb", [(ET - KY8) * 128, DIM], BF, kind="ExternalInput")
    dram("wout8", [KY8 * 128, DIM], F8, kind="ExternalInput")
    dram("wvb", [1, HID], BF, kind="ExternalInput")
    dram("bout", [1, DIM], BF, kind="ExternalInput")
    dram("bgate", [128, ET], F32, kind="ExternalInput")
    dram("bqk", [128, 1], F32, kind="ExternalInput")
    dram("triu", [128, 128], BF, kind="ExternalInput")
    dram("bdiag", [128, 128], BF, kind="ExternalInput")
    dram("bcorn", [128, 128], BF, kind="ExternalInput")
    dram("bprev", [32, 256], BF, kind="ExternalInput")
    dram("hmask", [32, 1], F32, kind="ExternalInput")
    if DEBUG_DUMPS:
        dram("dbg_qkT", [128, 4, T], BF, kind="ExternalOutput")
        dram("dbg_vh", [128, TT, HID], F8, kind="ExternalOutput")
        dram("dbg_outT", [128, ET, T], BF, kind="ExternalOutput")
        dram("dbg_sfull", [128, NG, HID], BF, kind="ExternalOutput")
    dram("cc_warm_in", [128, 16], BF)
    dram("cc_warm_out", [NC * 128, 16], BF, addr_space="Shared")
    dram("wsumw", [128, NC], F32, kind="ExternalInput")
    dram("cc_in0", [128, EH], F8)
    dram("cc_out0", [NC * 128, EH], F8, addr_space="Shared")
    dram("cc_in1", [128, EH], F8)
    dram("cc_out1", [NC * 128, EH], F8, addr_space="Shared")
    dram("y", [4, T, DIM], F32, kind="ExternalOutput")

    with tile.TileContext(nc) as tc:
        _emit(tc, ap)
    nc.compile()
    return nc


def _f8(a):
    return np.clip(a, -240.0, 240.0).astype(fp8)


def host_prep(inputs):
    """Pure layout transforms: shard, transpose, cast, build conv-band consts."""
    x = np.ascontiguousarray(np.asarray(inputs["x"], np.float32)[0])  # [4, N, DIM]
    W_h = np.asarray(inputs["W_h"], np.float32)
    b_h = np.asarray(inputs["b_h"], np.float32)
    W_qk = np.asarray(inputs["W_qk"], np.float32)
    b_qk = np.asarray(inputs["b_qk"], np.float32)
    W_out = np.asarray(inputs["W_out"], np.float32)
    b_out = np.asarray(inputs["b_out"], np.float32)
    cw = np.asarray(inputs["conv_w"], np.float32)

    jj = np.arange(128)[:, None]
    ii = np.arange(128)[None, :]
    d = ii - jj
    triu = (ii >= jj).astype(bf16)
    bdiag = np.where((d >= 0) & (d <= 31), cw[np.clip(31 - d, 0, 62)], 0.0).astype(bf16)
    dc = (ii + 128) - jj
    bcorn = np.where((dc >= 0) & (dc <= 31),
                     cw[np.clip(31 - dc, 0, 62)], 0.0).astype(bf16)
    jt = np.arange(32)[:, None]
    ip = np.arange(32)[None, :]
    dp = ip + 32 - jt
    bprev = np.zeros((32, 256), np.float32)
    bprev[:, :32] = np.where((dp >= 1) & (dp <= 31),
                             cw[np.clip(31 - dp, 0, 62)], 0.0)
    bprev = bprev.astype(bf16)

    kb0 = KG8 * 128
    ky0 = KY8 * 128
    common = {
        "wv8": _f8(W_h[:, :HID] * WS),
        "wgb": np.ascontiguousarray(W_h[kb0:, HID:]).astype(bf16),
        "wg8": _f8(W_h[:kb0, HID:] * WS),
        "wqk8": _f8(W_qk * WS),
        "woutb": np.ascontiguousarray(W_out[ky0:, :]).astype(bf16),
        "wout8": _f8(W_out[:ky0, :] * WS),
        "wvb": b_h[None, :HID].astype(bf16),
        "bout": b_out[None, :].astype(bf16),
        "bgate": np.ascontiguousarray(b_h[HID:].reshape(ET, 128).T).astype(np.float32),
        "bqk": b_qk[:, None].astype(np.float32),
        "triu": triu, "bdiag": bdiag, "bcorn": bcorn, "bprev": bprev,
    }

    in_maps = []
    for c in range(NC):
        sl = slice(c * T, (c + 1) * T)
        x_c = x[:, sl, :]
        xtb = np.zeros((4, KB * 128, T), bf16)
        xt8 = np.zeros((3, DIM, T), fp8)
        for s in range(4):
            xT = x_c[s].T
            xtb[s] = xT[kb0:].astype(bf16)
            if s > 0:
                xt8[s - 1] = _f8(xT * XS)
        xt08 = _f8(x_c[0].T * XS)
        if c > 0:
            xh8 = _f8(np.ascontiguousarray(x[0, c * T - 32:c * T, :].T) * XS)
        else:
            xh8 = np.zeros((DIM, 32), fp8)
        m = dict(common)
        m["xtb"] = xtb
        m["xt8"] = xt8
        m["xt08"] = xt08
        m["xh8"] = xh8
        m["xtok"] = np.ascontiguousarray(x_c)
        m["hmask"] = np.full((32, 1), 1.0 if c > 0 else 0.0, np.float32)
        w = np.zeros((128, NC), np.float32)
        w[:, :c] = 1.0
        m["wsumw"] = w
        in_maps.append(m)
    return in_maps


_NC_PROG = None
_NC_FLAGS = None


def kernel(**inputs):
    global _NC_PROG, _NC_FLAGS
    b_h = np.asarray(inputs["b_h"], np.float32)
    b_out = np.asarray(inputs["b_out"], np.float32)
    flags = (bool(np.any(b_h[:HID])), bool(np.any(b_out)))
    if _NC_PROG is None or _NC_FLAGS != flags:
        _NC_PROG = build_nc(with_vbias=flags[0], with_obias=flags[1])
        _NC_FLAGS = flags
    in_maps = host_prep(inputs)
    res = run_bass_kernel_spmd(_NC_PROG, in_maps, list(range(NC)))
    y = np.stack([res.results[c]["y"] for c in range(NC)], axis=1)  # [4, NC, T, DIM]
    return np.ascontiguousarray(y.reshape(4, NSEQ, DIM)[None]).astype(np.float32)


# revision 30
# speedup vs baseline: 1.1271x; 1.0305x over previous
"""Trainium2 Bass kernel for nn_FLASH_40458591928592 (sparse_attention).

Sequence-sharded over 8 NeuronCores: 1024 tokens (= 4 groups of 256) per core.
Mixed precision, validated against a numpy e4m3 simulation (rel 1.66e-2 < 2e-2):
  qk GEMM : fully fp8 DoubleRow (x*0.25 stationary-free scales, W*4)
  v GEMM  : fully fp8 DoubleRow; v_h and tails stored fp8 (storage only)
  gate    : k-tiles 0-1 of 8 fp8 DR, rest bf16 (same PSUM, products at scale 1)
  y       : HID k-tiles 0-3 of 16 fp8 DR (og et 0-3 written fp8*0.25 by DVE)

Phase order keeps the PE dense and the HAM clock warm:
  dummy warmup matmuls through the DMA lead-in -> v -> qk(3,2,0,1) -> sim/attn
  -> lk transposes -> kv chains (AllGather per e-half fires ~60us) -> quad+conv
  (overlapping the collectives) -> lin -> gate+y per stream.
One LDWEIGHTS feeds 2-4 matmuls everywhere (ec/ch/th/nch pairing).
SBUF is tag-chained across serial phases (vh->ogb, wv8->woutb, qkT->gt,
S_full->og8, t_half->wg8, lk->wout8, S_offb->xr, tails->ysb).
"""

from contextlib import ExitStack

import numpy as np
import ml_dtypes

import concourse.tile as tile
from concourse import bacc, mybir
from concourse.bass_utils import run_bass_kernel_spmd
from concourse.masks import make_identity

BF = mybir.dt.bfloat16
F8 = mybir.dt.float8e4
F32 = mybir.dt.float32
bf16 = ml_dtypes.bfloat16
fp8 = ml_dtypes.float8_e4m3
DRM = mybir.MatmulPerfMode.DoubleRow

G = 256
DIM = 1024
HID = 2048
DQK = 128
NSEQ = 8192
NC = 8
T = NSEQ // NC        # 1024 tokens per core
NG = T // G           # 4 groups per core
KD = DIM // 128       # 8 k-tiles over dim
ET = HID // 128       # 16 e-tiles over hid
TT = T // 128         # 8 token tiles
EH = HID // 2         # 1024 cols per e-half

KG8 = 2               # gate fp8 k-tiles (of KD); one DR pair
KY8 = 4               # y fp8 k-tiles (of ET); must be even
KB = KD - KG8         # bf16 k-tiles for gate
XS = 0.25             # fp8 x-side scale
WS = 4.0              # fp8 w-side scale (XS*WS == 1 -> shared-PSUM)
NWARM = 18            # HAM warmup dummy matmuls

AF = mybir.ActivationFunctionType
ALU = mybir.AluOpType

DEBUG_DUMPS = False
WITH_VBIAS = True
WITH_OBIAS = True


def _emit(tc, ap):
    nc = tc.nc
    with ExitStack() as ctx:
        pass

        consts = ctx.enter_context(tc.tile_pool(name="consts", bufs=1))
        p_x8 = ctx.enter_context(tc.tile_pool(name="x8", bufs=2))
        p_xtb = ctx.enter_context(tc.tile_pool(name="xtb", bufs=3))
        p_x08 = ctx.enter_context(tc.tile_pool(name="x08", bufs=1))
        p_big = ctx.enter_context(tc.tile_pool(name="big", bufs=1))   # vh8 -> ogb
        p_qog = ctx.enter_context(tc.tile_pool(name="qog", bufs=1))   # qkT -> gt
        p_lk = ctx.enter_context(tc.tile_pool(name="lk", bufs=1))     # lk -> wout8
        p_w1 = ctx.enter_context(tc.tile_pool(name="w1", bufs=1))     # wv8 -> woutb
        p_w2 = ctx.enter_context(tc.tile_pool(name="w2", bufs=1))     # wgb
        p_tails = ctx.enter_context(tc.tile_pool(name="tails", bufs=1))  # -> ysb
        p_so = ctx.enter_context(tc.tile_pool(name="so", bufs=2))     # t_half -> wg8
        p_sob = ctx.enter_context(tc.tile_pool(name="sob", bufs=2))   # xr
        p_sf = ctx.enter_context(tc.tile_pool(name="sf", bufs=1))     # S_full -> og8
        p_tr = ctx.enter_context(tc.tile_pool(name="tr", bufs=4))
        p_a0 = ctx.enter_context(tc.tile_pool(name="a0", bufs=4))
        p_a1 = ctx.enter_context(tc.tile_pool(name="a1", bufs=4))
        p_outT = ctx.enter_context(tc.tile_pool(name="outT", bufs=1))
        ps1 = ctx.enter_context(tc.tile_pool(name="ps1", bufs=3, space="PSUM"))
        ps2 = ctx.enter_context(tc.tile_pool(name="ps2", bufs=2, space="PSUM"))

        # warm-up collective first: its ~40us post-trigger setup runs in the
        # shadow of the input DMAs, so the real AllGathers start immediately
        cwarm = consts.tile([128, 16], BF, tag="cwarm")
        nc.vector.memset(cwarm, 0.0)
        nc.sync.dma_start(ap["cc_warm_in"], cwarm)
        nc.gpsimd.collective_compute(
            "AllGather", ALU.bypass, replica_groups=[list(range(NC))],
            ins=[ap["cc_warm_in"]], outs=[ap["cc_warm_out"]])

        # ---- HAM warmup: keep PE busy through the DMA lead-in ----
        ident = consts.tile([128, 128], BF, tag="ident")
        make_identity(nc, ident)
        for _ in range(NWARM):
            pw = ps2.tile([128, 128], F32, tag="ps2", name="pw")
            nc.tensor.matmul(pw, ident, ident, start=True, stop=True)

        # ---- first DMAs: v-GEMM inputs, then qk weights ----
        x08 = p_x08.tile([128, KD, T], F8, tag="x08")
        wv8 = p_w1.tile([128, KD, HID], F8, tag="w1")
        for q in range(4):
            nc.sync.dma_start(
                x08[:, q * 2:(q + 1) * 2, :],
                ap["xt08"][q * 256:(q + 1) * 256, :].rearrange(
                    "(kt p) t -> p kt t", p=128))
            eng = nc.scalar if q < 2 else nc.gpsimd
            eng.dma_start(
                wv8[:, q * 2:(q + 1) * 2, :],
                ap["wv8"][q * 256:(q + 1) * 256, :].rearrange(
                    "(kt p) e -> p kt e", p=128))
        bqk = consts.tile([128, 1], F32, tag="bqk")
        nc.scalar.dma_start(bqk, ap["bqk"])
        wqk8 = consts.tile([128, KD, DQK], F8, tag="wqk8")
        nc.scalar.dma_start(wqk8, ap["wqk8"].rearrange("(kt p) q -> p kt q", p=128))
        xh8 = consts.tile([128, KD, 32], F8, tag="xh8")
        nc.scalar.dma_start(xh8, ap["xh8"].rearrange("(kt p) t -> p kt t", p=128))

        x8f = {0: x08}

        def load_x8f(s):
            t8 = p_x8.tile([128, KD, T], F8, tag="x8", name=f"x8_{s}")
            nc.sync.dma_start(t8, ap["xt8"][s - 1].rearrange("(kt p) t -> p kt t",
                                                             p=128))
            x8f[s] = t8

        load_x8f(3)
        load_x8f(2)

        # remaining consts (DMA behind weights on scalar queue)
        triu = consts.tile([128, 128], BF, tag="triu")
        nc.scalar.dma_start(triu, ap["triu"])
        bdiag = consts.tile([128, 128], BF, tag="bdiag")
        nc.scalar.dma_start(bdiag, ap["bdiag"])
        bcorn = consts.tile([128, 128], BF, tag="bcorn")
        nc.scalar.dma_start(bcorn, ap["bcorn"])
        bprev = consts.tile([32, 256], BF, tag="bprev")
        nc.scalar.dma_start(bprev, ap["bprev"])
        hmask = consts.tile([32, 1], F32, tag="hmask")
        nc.scalar.dma_start(hmask, ap["hmask"])
        wsumw = consts.tile([128, NC], F32, tag="wsumw")
        nc.scalar.dma_start(wsumw, ap["wsumw"])
        bgate = consts.tile([128, ET], F32, tag="bgate")
        nc.scalar.dma_start(bgate, ap["bgate"])
        if WITH_VBIAS or WITH_OBIAS:
            ones_t = consts.tile([1, 1024], BF, tag="ones")
            nc.vector.memset(ones_t, 1.0)
        if WITH_VBIAS:
            wvb = consts.tile([1, HID], BF, tag="wvb")
            nc.scalar.dma_start(wvb, ap["wvb"])
        if WITH_OBIAS:
            bout = consts.tile([1, DIM], BF, tag="bout")
            nc.scalar.dma_start(bout, ap["bout"])

        # ---- v GEMM: fp8 DR, one xt-pair LDWEIGHTS feeds 4 e-chunks ----
        v_h = p_big.tile([128, TT, HID], F8, tag="big", name="v_h")
        for tt in range(TT):
            pv = [ps1.tile([128, 1024], F32, tag="ps1", name="pv")
                  for _ in range(2)]
            for kp in range(KD // 2):
                for ec in range(4):
                    nc.tensor.matmul(
                        pv[ec // 2][:, (ec % 2) * 512:(ec % 2 + 1) * 512],
                        x08[:, 2 * kp:2 * kp + 2, tt * 128:(tt + 1) * 128],
                        wv8[:, 2 * kp:2 * kp + 2, ec * 512:(ec + 1) * 512],
                        start=(kp == 0),
                        stop=(kp == KD // 2 - 1 and not WITH_VBIAS),
                        perf_mode=DRM, skip_group_check=True)
            if WITH_VBIAS:
                for ec in range(4):
                    nc.tensor.matmul(pv[ec // 2][:, (ec % 2) * 512:(ec % 2 + 1) * 512],
                                     ones_t[0:1, 0:128],
                                     wvb[0:1, ec * 512:(ec + 1) * 512],
                                     start=False, stop=True, skip_group_check=True)
            for eh in range(2):
                nc.scalar.activation(v_h[:, tt, eh * 1024:(eh + 1) * 1024], pv[eh],
                                     AF.Silu, bias=0.0, scale=1.0)

        # halo: last 32 tokens of the previous core (masked on core 0)
        tails = p_tails.tile([32, NG, HID], F8, tag="tails")
        ph = [ps1.tile([32, 1024], F32, tag="ps1", name="ph") for _ in range(2)]
        for kp in range(KD // 2):
            for ec in range(4):
                nc.tensor.matmul(
                    ph[ec // 2][:, (ec % 2) * 512:(ec % 2 + 1) * 512],
                    xh8[:, 2 * kp:2 * kp + 2, :],
                    wv8[:, 2 * kp:2 * kp + 2, ec * 512:(ec + 1) * 512],
                    start=(kp == 0), stop=(kp == KD // 2 - 1 and not WITH_VBIAS),
                    perf_mode=DRM, skip_group_check=True)
        if WITH_VBIAS:
            for ec in range(4):
                nc.tensor.matmul(ph[ec // 2][:, (ec % 2) * 512:(ec % 2 + 1) * 512],
                                 ones_t[0:1, 0:32],
                                 wvb[0:1, ec * 512:(ec + 1) * 512],
                                 start=False, stop=True, skip_group_check=True)
        for eh in range(2):
            nc.scalar.activation(tails[:, 0, eh * 1024:(eh + 1) * 1024], ph[eh],
                                 AF.Silu, bias=0.0, scale=1.0)
            nc.vector.tensor_scalar_mul(tails[:, 0, eh * 1024:(eh + 1) * 1024],
                                        tails[:, 0, eh * 1024:(eh + 1) * 1024],
                                        hmask)
        for g in range(1, NG):
            nc.scalar.dma_start(tails[:, g, :], v_h[96:128, 2 * g - 1, :])

        # ---- qk streams: fully fp8 DR, ch-paired ----
        qkT = p_qog.tile([128, 4, T], BF, tag="qog", name="qkT")

        def qk_stream(s):
            pc = ps1.tile([128, 1024], F32, tag="ps1", name="pc")
            for kp in range(KD // 2):
                for ch in range(2):
                    nc.tensor.matmul(pc[:, ch * 512:(ch + 1) * 512],
                                     wqk8[:, 2 * kp:2 * kp + 2, :],
                                     x8f[s][:, 2 * kp:2 * kp + 2,
                                            ch * 512:(ch + 1) * 512],
                                     start=(kp == 0), stop=(kp == KD // 2 - 1),
                                     perf_mode=DRM, skip_group_check=True)
            nc.scalar.activation(qkT[:, s, :], pc, AF.Silu, bias=bqk, scale=1.0)

        qk_stream(3)
        load_x8f(1)   # slot rotation WARs on qk3's reads

        # lk (stream 3) token-major via PE transpose
        lk_tok = p_lk.tile([128, TT, 128], BF, tag="lk", name="lk_tok")
        for tt in range(TT):
            pt = ps2.tile([128, 128], BF, tag="ps2", name="pt")
            nc.tensor.transpose(pt, qkT[:, 3, tt * 128:(tt + 1) * 128], ident)
            nc.vector.tensor_copy(lk_tok[:, tt, :], pt)

        # ---- kv chains + AllGather per e-half ----
        S_full = p_sf.tile([128, NG, HID], BF, tag="sf", name="S_full")

        def wsum_half(eh):
            e0 = eh * EH
            cc_out = ap[f"cc_out{eh}"]
            for r in range(NC):
                for hh in range(2):
                    s0 = S_full[:, 0, e0 + hh * 512:e0 + (hh + 1) * 512]
                    tr = p_tr.tile([128, 512], F8, tag="tr")
                    nc.sync.dma_start(
                        tr, cc_out[r * 128:(r + 1) * 128,
                                   hh * 512:(hh + 1) * 512])
                    if r == 0:
                        nc.vector.tensor_scalar_mul(s0, tr, wsumw[:, 0:1])
                    else:
                        nc.vector.scalar_tensor_tensor(
                            s0, tr, wsumw[:, r:r + 1], s0,
                            op0=ALU.mult, op1=ALU.add)
            for g in range(1, NG):
                nc.vector.tensor_add(S_full[:, g, e0:e0 + EH],
                                     S_full[:, g, e0:e0 + EH],
                                     S_full[:, 0, e0:e0 + EH])

        # gate inputs/weights early: scalar ring ahead of the cc_in triggers,
        # sync ring ahead of the wsum tr reads
        wgb = p_w2.tile([128, KB, HID], BF, tag="w2", name="wgb")
        for kt in range(KB):
            nc.scalar.dma_start(wgb[:, kt, :],
                                ap["wgb"][kt * 128:(kt + 1) * 128, :])
        xtb = {}

        def load_xtb(s):
            halves = []
            for q in range(2):
                h = p_xtb.tile([128, KB // 2, T], BF, tag="xtb", name=f"xtb{s}_{q}")
                nc.sync.dma_start(
                    h, ap["xtb"][s, q * 384:(q + 1) * 384, :].rearrange(
                        "(kt p) t -> p kt t", p=128))
                halves.append(h)
            xtb[s] = halves

        load_xtb(0)

        for eh in range(2):
            e0 = eh * EH
            t_half = p_so.tile([128, EH], F8, tag="so", name="t_half")
            for g in range(NG):
                pk = ps1.tile([128, 1024], F32, tag="ps1", name="pk")
                for jt in range(2):
                    for ec in range(2):
                        nc.tensor.matmul(
                            pk[:, ec * 512:(ec + 1) * 512],
                            lk_tok[:, 2 * g + jt, :],
                            v_h[:, 2 * g + jt, e0 + ec * 512:e0 + (ec + 1) * 512],
                            start=(jt == 0), stop=(jt == 1),
                            skip_group_check=True)
                dst = (S_full[:, g + 1, e0:e0 + EH] if g < NG - 1 else t_half)
                nc.scalar.activation(dst, pk, AF.Copy, bias=0.0, scale=1.0 / G)
            # exclusive-prefix over local groups on DVE (off the PE path)
            for g in range(2, NG):
                nc.vector.tensor_add(S_full[:, g, e0:e0 + EH],
                                     S_full[:, g, e0:e0 + EH],
                                     S_full[:, g - 1, e0:e0 + EH])
            nc.vector.tensor_add(t_half, t_half, S_full[:, NG - 1, e0:e0 + EH])
            # fire the AllGather; the weighted sum is emitted after attn so
            # the DVE stream is not blocked waiting on the collective
            cc_in, cc_out = ap[f"cc_in{eh}"], ap[f"cc_out{eh}"]
            nc.scalar.dma_start(cc_in, t_half)
            nc.gpsimd.collective_compute(
                "AllGather", ALU.bypass, replica_groups=[list(range(NC))],
                ins=[cc_in], outs=[cc_out])

        for s in (2, 0, 1):
            qk_stream(s)

        # ---- sim/attn per group (conv band folded into bdiag/bcorn) ----
        attn0, attn1 = [], []
        for g in range(NG):
            i0 = g * G
            a0 = p_a0.tile([128, 256], BF, tag="a0")
            ps = ps2.tile([128, 256], F32, tag="ps2")
            nc.tensor.matmul(ps, qkT[:, 2, i0:i0 + 128], qkT[:, 0, i0:i0 + 256],
                             start=True, stop=True)
            nc.scalar.activation(a0, ps, AF.Relu, bias=0.0, scale=1.0 / G)
            nc.vector.tensor_mul(a0[:, 0:128], a0[:, 0:128], triu)
            nc.vector.tensor_mul(a0, a0, a0)
            nc.vector.tensor_add(a0[:, 0:128], a0[:, 0:128], bdiag)
            nc.vector.tensor_add(a0[:, 128:256], a0[:, 128:256], bcorn)
            attn0.append(a0)

            a1 = p_a1.tile([128, 256], BF, tag="a1")
            nc.vector.memset(a1[:, 0:128], 0.0)
            ps = ps2.tile([128, 256], F32, tag="ps2")
            nc.tensor.matmul(ps[:, 0:128], qkT[:, 2, i0 + 128:i0 + 256],
                             qkT[:, 0, i0 + 128:i0 + 256], start=True, stop=True)
            a1r = a1[:, 128:256]
            nc.scalar.activation(a1r, ps[:, 0:128], AF.Relu, bias=0.0, scale=1.0 / G)
            nc.vector.tensor_mul(a1r, a1r, triu)
            nc.vector.tensor_mul(a1r, a1r, a1r)
            nc.vector.tensor_add(a1r, a1r, bdiag)
            attn1.append(a1)


        wsum_half(0)
        wsum_half(1)

        # ---- quad + conv boundary -> outT ----
        outT = p_outT.tile([128, ET, T], BF, tag="outT")
        pdum = ps2.tile([128, 512], F32, tag="ps2", name="pdum")
        for eh in range(2):
            e0 = eh * EH
            for et in range(8):
                ec0 = e0 + et * 128
                po = ps1.tile([128, 1024], F32, tag="ps1", name="po")
                for g in range(NG):
                    c0 = g * G
                    if g % 2 == 0:
                        nc.tensor.matmul(pdum, ident, qkT[:, 0, 0:512],
                                         start=True, stop=True,
                                         skip_group_check=True)
                    nc.tensor.matmul(po[:, c0:c0 + 256],
                                     v_h[:, 2 * g, ec0:ec0 + 128], attn0[g],
                                     start=True, stop=False, skip_group_check=True)
                    nc.tensor.matmul(po[:, c0:c0 + 256],
                                     v_h[:, 2 * g + 1, ec0:ec0 + 128],
                                     attn1[g], start=False, stop=False,
                                     skip_group_check=True)
                    nc.tensor.matmul(po[:, c0:c0 + 256],
                                     tails[:, g, ec0:ec0 + 128], bprev,
                                     start=False, stop=True, skip_group_check=True)
                if et % 2 == 0:
                    nc.scalar.activation(outT[:, eh * 8 + et, :], po,
                                         AF.Copy, bias=0.0, scale=1.0)
                else:
                    nc.vector.tensor_copy(outT[:, eh * 8 + et, :], po)

        def gate_unit(s, et, og8, ogb, defer_mul=False):
            pg = ps1.tile([128, 1024], F32, tag="ps1", name="pg")
            for th in range(2):
                nc.tensor.matmul(
                    pg[:, th * 512:(th + 1) * 512],
                    wg8[:, 0:KG8, et * 128:(et + 1) * 128],
                    x8f[s][:, 0:KG8, th * 512:(th + 1) * 512],
                    start=True, stop=False,
                    perf_mode=DRM, skip_group_check=True)
            for kt in range(KB):
                xs_t = xtb[s][kt // 3][:, kt % 3, :]
                for th in range(2):
                    nc.tensor.matmul(
                        pg[:, th * 512:(th + 1) * 512],
                        wgb[:, kt, et * 128:(et + 1) * 128],
                        xs_t[:, th * 512:(th + 1) * 512],
                        start=False, stop=(kt == KB - 1),
                        skip_group_check=True)
            if et < KY8:
                gt = p_qog.tile([128, 1024], BF, tag="qog", name="gt")
                nc.scalar.activation(gt, pg, AF.Silu,
                                     bias=bgate[:, et:et + 1], scale=1.0)
                nc.vector.scalar_tensor_tensor(
                    og8[:, et, :], gt, XS, outT[:, et, :],
                    op0=ALU.mult, op1=ALU.mult)
            else:
                eb = et - KY8
                nc.scalar.activation(ogb[:, eb, :], pg, AF.Silu,
                                     bias=bgate[:, et:et + 1], scale=1.0)
                if not defer_mul:
                    nc.vector.tensor_mul(ogb[:, eb, :], ogb[:, eb, :],
                                         outT[:, et, :])

        wg8 = p_so.tile([128, KG8, HID], F8, tag="so", name="wg8")
        nc.scalar.dma_start(wg8, ap["wg8"].rearrange("(kt p) e -> p kt e", p=128))

        # stream-0 gate units prefetched into the AllGather/wsum waits
        # (their og-multiplies are deferred until outT is final after lin)
        PRE_ETS = (4, 5, 6, 7, 8, 9, 10, 11)
        ogb0 = p_big.tile([128, ET - KY8, T], BF, tag="big", name="ogb")
        for et in PRE_ETS[:4]:
            gate_unit(0, et, None, ogb0, defer_mul=True)

        # scheduler fence: without it the scheduler hoists the lin matmuls
        # (which wait on the AllGather+wsum) ahead of quad and parks the PE
        tc.no_sync_barrier()

        # ---- lin joined via DVE add into outT ----
        def lin_half(eh):
            for et in range(eh * 8, eh * 8 + 8):
                po = ps1.tile([128, 1024], F32, tag="ps1", name="po")
                for g in range(NG):
                    if g % 2 == 0:
                        nc.tensor.matmul(pdum, ident, qkT[:, 0, 0:512],
                                         start=True, stop=True,
                                         skip_group_check=True)
                    nc.tensor.matmul(po[:, g * G:(g + 1) * G],
                                     S_full[:, g, et * 128:(et + 1) * 128],
                                     qkT[:, 1, g * G:(g + 1) * G],
                                     start=True, stop=True, skip_group_check=True)
                nc.vector.tensor_add(outT[:, et, :], outT[:, et, :], po)

        lin_half(0)
        for et in PRE_ETS[4:]:
            gate_unit(0, et, None, ogb0, defer_mul=True)
        tc.no_sync_barrier()
        lin_half(1)

        # out-projection weights (DMA during lin/first gate)
        woutb = p_w1.tile([128, ET - KY8, DIM], BF, tag="w1", name="woutb")
        for kt in range(ET - KY8):
            nc.scalar.dma_start(woutb[:, kt, :],
                                ap["woutb"][kt * 128:(kt + 1) * 128, :])
        wout8 = p_lk.tile([128, KY8, DIM], F8, tag="lk", name="wout8")
        nc.scalar.dma_start(wout8, ap["wout8"].rearrange("(kt p) n -> p kt n",
                                                         p=128))

        if DEBUG_DUMPS:
            nc.sync.dma_start(ap["dbg_qkT"], qkT)
            nc.sync.dma_start(ap["dbg_vh"], v_h)
            nc.sync.dma_start(ap["dbg_outT"], outT)
            nc.sync.dma_start(ap["dbg_sfull"], S_full)

        load_xtb(1)

        # ---- gate + y per stream (th-paired gate, nch-paired y) ----
        for s in range(4):
            og8 = p_sf.tile([128, KY8, T], F8, tag="sf", name="og8")
            ogb = ogb0 if s == 0 else p_big.tile([128, ET - KY8, T], BF,
                                                 tag="big", name="ogb")
            for et in range(ET):
                if s == 0 and et in PRE_ETS:
                    eb = et - KY8
                    nc.vector.tensor_mul(ogb[:, eb, :], ogb[:, eb, :],
                                         outT[:, et, :])
                    continue
                gate_unit(s, et, og8, ogb)
            if s < 2:
                load_xtb(s + 2)
            if s == 0:
                load_x8f(1)
            elif s == 1:
                load_x8f(2)
            elif s == 2:
                load_x8f(3)
            for tl in range(TT):
                tsl = slice(tl * 128, (tl + 1) * 128)
                pn = ps1.tile([128, 1024], F32, tag="ps1", name="pn")
                for kp in range(KY8 // 2):
                    for nch in range(2):
                        nc.tensor.matmul(
                            pn[:, nch * 512:(nch + 1) * 512],
                            og8[:, 2 * kp:2 * kp + 2, tsl],
                            wout8[:, 2 * kp:2 * kp + 2,
                                  nch * 512:(nch + 1) * 512],
                            start=(kp == 0), stop=False,
                            perf_mode=DRM, skip_group_check=True)
                for kt in range(ET - KY8):
                    for nch in range(2):
                        nc.tensor.matmul(
                            pn[:, nch * 512:(nch + 1) * 512],
                            ogb[:, kt, tsl],
                            woutb[:, kt, nch * 512:(nch + 1) * 512],
                            start=False,
                            stop=(kt == ET - KY8 - 1 and not WITH_OBIAS),
                            skip_group_check=True)
                if WITH_OBIAS:
                    for nch in range(2):
                        nc.tensor.matmul(pn[:, nch * 512:(nch + 1) * 512],
                                         ones_t[0:1, 0:128],
                                         bout[0:1, nch * 512:(nch + 1) * 512],
                                         start=False, stop=True,
                                         skip_group_check=True)
                xr = p_sob.tile([128, 1024], F32, tag="xr", name="xr")
                nc.sync.dma_start(xr, ap["xtok"][s, tsl, :])
                ysb = p_x08.tile([128, 1024], F32, tag="x08", name="ysb")
                nc.vector.scalar_tensor_tensor(
                    ysb, pn, 0.0, xr, op0=ALU.add, op1=ALU.add)
                nc.sync.dma_start(ap["y"][s, tsl, :], ysb)


def build_nc(with_vbias=None, with_obias=None):
    global WITH_VBIAS, WITH_OBIAS
    if with_vbias is not None:
        WITH_VBIAS = with_vbias
    if with_obias is not None:
        WITH_OBIAS = with_obias
    nc = bacc.Bacc("TRN2", target_bir_lowering=False, debug=False, num_devices=NC)
    ap = {}

    def dram(name, shape, dt, kind=None, addr_space=None):
        kw = {}
        if kind:
            kw["kind"] = kind
        if addr_space:
            kw["addr_space"] = addr_space
        ap[name] = nc.dram_tensor(name, shape, dt, **kw).ap()

    dram("xtb", [4, KB * 128, T], BF, kind="ExternalInput")
    dram("xt8", [3, DIM, T], F8, kind="ExternalInput")
    dram("xt08", [DIM, T], F8, kind="ExternalInput")
    dram("xh8", [DIM, 32], F8, kind="ExternalInput")
    dram("xtok", [4, T, DIM], F32, kind="ExternalInput")
    dram("wv8", [DIM, HID], F8, kind="ExternalInput")
    dram("wgb", [KB * 128, HID], BF, kind="ExternalInput")
    dram("wg8", [KG8 * 128, HID], F8, kind="ExternalInput")
    dram("wqk8", [DIM, DQK], F8, kind="ExternalInput")
    dram("woutb", [(ET - KY8) * 128, DIM], BF, kind="ExternalInput")
    dram("wout8", [KY8 * 128, DIM], F8, kind="ExternalInput")
    dram("wvb", [1, HID], BF, kind="ExternalInput")
    dram("bout", [1, DIM], BF, kind="ExternalInput")
    dram("bgate", [128, ET], F32, kind="ExternalInput")
    dram("bqk", [128, 1], F32, kind="ExternalInput")
    dram("triu", [128, 128], BF, kind="ExternalInput")
    dram("bdiag", [128, 128], BF, kind="ExternalInput")
    dram("bcorn", [128, 128], BF, kind="ExternalInput")
    dram("bprev", [32, 256], BF, kind="ExternalInput")
    dram("hmask", [32, 1], F32, kind="ExternalInput")
    if DEBUG_DUMPS:
        dram("dbg_qkT", [128, 4, T], BF, kind="ExternalOutput")
        dram("dbg_vh", [128, TT, HID], F8, kind="ExternalOutput")
        dram("dbg_outT", [128, ET, T], BF, kind="ExternalOutput")
        dram("dbg_sfull", [128, NG, HID], BF, kind="ExternalOutput")
    dram("cc_warm_in", [128, 16], BF)
    dram("cc_warm_out", [NC * 128, 16], BF, addr_space="Shared")
    dram("wsumw", [128, NC], F32, kind="ExternalInput")
    dram("cc_in0", [128, EH], F8)
    dram("cc_out0", [NC * 128, EH], F8, addr_space="Shared")
    dram("cc_in1", [128, EH], F8)
    dram("cc_out1", [NC * 128, EH], F8, addr_space="Shared")
    dram("y", [4, T, DIM], F32, kind="ExternalOutput")

    with tile.TileContext(nc) as tc:
        _emit(tc, ap)
    nc.compile()
    return nc


def _f8(a):
    return np.clip(a, -240.0, 240.0).astype(fp8)


def host_prep(inputs):
    """Pure layout transforms: shard, transpose, cast, build conv-band consts."""
    x = np.ascontiguousarray(np.asarray(inputs["x"], np.float32)[0])  # [4, N, DIM]
    W_h = np.asarray(inputs["W_h"], np.float32)
    b_h = np.asarray(inputs["b_h"], np.float32)
    W_qk = np.asarray(inputs["W_qk"], np.float32)
    b_qk = np.asarray(inputs["b_qk"], np.float32)
    W_out = np.asarray(inputs["W_out"], np.float32)
    b_out = np.asarray(inputs["b_out"], np.float32)
    cw = np.asarray(inputs["conv_w"], np.float32)

    jj = np.arange(128)[:, None]
    ii = np.arange(128)[None, :]
    d = ii - jj
    triu = (ii >= jj).astype(bf16)
    bdiag = np.where((d >= 0) & (d <= 31), cw[np.clip(31 - d, 0, 62)], 0.0).astype(bf16)
    dc = (ii + 128) - jj
    bcorn = np.where((dc >= 0) & (dc <= 31),
                     cw[np.clip(31 - dc, 0, 62)], 0.0).astype(bf16)
    jt = np.arange(32)[:, None]
    ip = np.arange(32)[None, :]
    dp = ip + 32 - jt
    bprev = np.zeros((32, 256), np.float32)
    bprev[:, :32] = np.where((dp >= 1) & (dp <= 31),
                             cw[np.clip(31 - dp, 0, 62)], 0.0)
    bprev = bprev.astype(bf16)

    kb0 = KG8 * 128
    ky0 = KY8 * 128
    common = {
        "wv8": _f8(W_h[:, :HID] * WS),
        "wgb": np.ascontiguousarray(W_h[kb0:, HID:]).astype(bf16),
        "wg8": _f8(W_h[:kb0, HID:] * WS),
        "wqk8": _f8(W_qk * WS),
        "woutb": np.ascontiguousarray(W_out[ky0:, :]).astype(bf16),
        "wout8": _f8(W_out[:ky0, :] * WS),
        "wvb": b_h[None, :HID].astype(bf16),
        "bout": b_out[None, :].astype(bf16),
        "bgate": np.ascontiguousarray(b_h[HID:].reshape(ET, 128).T).astype(np.float32),
        "bqk": b_qk[:, None].astype(np.float32),
        "triu": triu, "bdiag": bdiag, "bcorn": bcorn, "bprev": bprev,
    }

    in_maps = []
    for c in range(NC):
        sl = slice(c * T, (c + 1) * T)
        x_c = x[:, sl, :]
        xtb = np.zeros((4, KB * 128, T), bf16)
        xt8 = np.zeros((3, DIM, T), fp8)
        for s in range(4):
            xT = x_c[s].T
            xtb[s] = xT[kb0:].astype(bf16)
            if s > 0:
                xt8[s - 1] = _f8(xT * XS)
        xt08 = _f8(x_c[0].T * XS)
        if c > 0:
            xh8 = _f8(np.ascontiguousarray(x[0, c * T - 32:c * T, :].T) * XS)
        else:
            xh8 = np.zeros((DIM, 32), fp8)
        m = dict(common)
        m["xtb"] = xtb
        m["xt8"] = xt8
        m["xt08"] = xt08
        m["xh8"] = xh8
        m["xtok"] = np.ascontiguousarray(x_c)
        m["hmask"] = np.full((32, 1), 1.0 if c > 0 else 0.0, np.float32)
        w = np.zeros((128, NC), np.float32)
        w[:, :c] = 1.0
        m["wsumw"] = w
        in_maps.append(m)
    return in_maps


_NC_PROG = None
_NC_FLAGS = None


def kernel(**inputs):
    global _NC_PROG, _NC_FLAGS
    b_h = np.asarray(inputs["b_h"], np.float32)
    b_out = np.asarray(inputs["b_out"], np.float32)
    flags = (bool(np.any(b_h[:HID])), bool(np.any(b_out)))
    if _NC_PROG is None or _NC_FLAGS != flags:
        _NC_PROG = build_nc(with_vbias=flags[0], with_obias=flags[1])
        _NC_FLAGS = flags
    in_maps = host_prep(inputs)
    res = run_bass_kernel_spmd(_NC_PROG, in_maps, list(range(NC)))
    y = np.stack([res.results[c]["y"] for c in range(NC)], axis=1)  # [4, NC, T, DIM]
    return np.ascontiguousarray(y.reshape(4, NSEQ, DIM)[None]).astype(np.float32)
